# revision 32
# baseline (speedup 1.0000x reference)
"""Trainium2 Bass kernel for nn_Attention (B=2, T=2048, E=1024, H=16, D=64).

Sharding: 2 heads per core across 8 cores (tensor-parallel over heads).
Each core computes Q/K/V projections for its 2 heads, causal attention,
and a partial out-projection (its 128 feature columns of Wo); the host
sums the 8 partial outputs.

Rev2: all tile transposes (V and P) moved off the tensor engine onto the
DMA XBAR (dma_start_transpose), PV restructured to u-outer 512-col
streams over a u-major P^T layout, and PV matmuls of block tb-1
interleaved into block tb's QK phase so the tensor engine fills the
softmax-chain stalls. Chunk maxes split between DVE (h0) and Pool (h1).
"""

import os
import sys

sys.path.insert(0, "/opt/trn_rl_repo")

import numpy as np
import concourse.bass as bass
import concourse.mybir as mybir
import concourse.tile as tile
from concourse import bacc
from concourse import bass_utils
from concourse.masks import make_identity

f32 = mybir.dt.float32
fp16 = mybir.dt.float16
bf16 = mybir.dt.bfloat16
AF = mybir.ActivationFunctionType
ALU = mybir.AluOpType
AX = mybir.AxisListType

B, T, E, H, D = 2, 2048, 1024, 16, 64
HL = 2              # heads per core
F = HL * D          # local feature cols (128)
NT = T // 128       # 16 t-tiles per batch
NE = E // 128       # 8 e-tiles
NTB = T // 512      # 4 t-blocks per batch
N_CORES = 8
INV_S = 1.0 / float(np.sqrt(T))


def build_nc():
    nc = bacc.Bacc("TRN2", target_bir_lowering=False, debug=False,
                   num_devices=N_CORES)
    xt_d = nc.dram_tensor("xt", [B, E, T], fp16, kind="ExternalInput").ap()
    wq_d = nc.dram_tensor("wq", [E, F], fp16, kind="ExternalInput").ap()
    wk_d = nc.dram_tensor("wk", [E, F], fp16, kind="ExternalInput").ap()
    wv_d = nc.dram_tensor("wv", [E, F], fp16, kind="ExternalInput").ap()
    wot_d = nc.dram_tensor("wot", [F, E], fp16, kind="ExternalInput").ap()
    sel8_d = nc.dram_tensor("sel8", [8, 4, 128], fp16,
                            kind="ExternalInput").ap()
    out_d = nc.dram_tensor("out", [B, T, E], bf16, kind="ExternalOutput").ap()

    with tile.TileContext(nc) as tc:
        with tc.tile_pool(name="const", bufs=1) as cpool, \
             tc.tile_pool(name="xtp", bufs=2) as xtp, \
             tc.tile_pool(name="qkv", bufs=2) as qkvp, \
             tc.tile_pool(name="pp", bufs=4) as ppool, \
             tc.tile_pool(name="ptb", bufs=2) as ptbp, \
             tc.tile_pool(name="smallp", bufs=8) as smallp, \
             tc.tile_pool(name="outp", bufs=6) as outp, \
             tc.tile_pool(name="ps_s", bufs=5, space="PSUM") as ps_s, \
             tc.tile_pool(name="ps_a", bufs=1, space="PSUM") as ps_a, \
             tc.tile_pool(name="ps_o", bufs=2, space="PSUM") as ps_o:

            # ---- constants ----
            ident_f = cpool.tile([128, 128], f32)
            make_identity(nc, ident_f[:])
            ident_h = cpool.tile([128, 128], fp16)
            nc.vector.tensor_copy(ident_h[:], ident_f[:])
            mask_f = cpool.tile([128, 128], f32)
            nc.gpsimd.memset(mask_f[:], 0.0)
            nc.gpsimd.affine_select(
                out=mask_f[:], in_=mask_f[:], compare_op=ALU.is_ge,
                fill=-30000.0, base=0, pattern=[[-1, 128]], channel_multiplier=1)
            mask_h = cpool.tile([128, 128], fp16)
            nc.vector.tensor_copy(mask_h[:], mask_f[:])
            # head-selector: sel[i, f] = 1 iff 64*i <= f < 64*i + 64
            sel_f = cpool.tile([2, 128], f32)
            nc.gpsimd.memset(sel_f[:], 1.0)
            nc.gpsimd.affine_select(
                out=sel_f[:], in_=sel_f[:], compare_op=ALU.is_ge, fill=0.0,
                base=0, pattern=[[1, 128]], channel_multiplier=-64)
            nc.gpsimd.affine_select(
                out=sel_f[:], in_=sel_f[:], compare_op=ALU.is_ge, fill=0.0,
                base=63, pattern=[[-1, 128]], channel_multiplier=64)
            sel_r = cpool.tile([2, 128], fp16)
            nc.vector.tensor_copy(sel_r[:], sel_f[:])

            # ---- weights ----
            wq_s = cpool.tile([128, NE, F], fp16)
            wk_s = cpool.tile([128, NE, F], fp16)
            wv_s = cpool.tile([128, NE, F], fp16)
            wot_s = cpool.tile([128, E], fp16)
            sel8_s = cpool.tile([8, 4, 128], fp16)
            nc.sync.dma_start(sel8_s[:], sel8_d)
            nc.sync.dma_start(wq_s[:], wq_d.rearrange("(n p) f -> p n f", p=128))
            nc.sync.dma_start(wk_s[:], wk_d.rearrange("(n p) f -> p n f", p=128))
            nc.sync.dma_start(wv_s[:], wv_d.rearrange("(n p) f -> p n f", p=128))
            nc.sync.dma_start(wot_s[:], wot_d)

            # deferred work queues: PV matmuls and per-tau normalize/
            # out-projection thunks of the previous t-block, interleaved
            # into later emission points so no engine sits idle
            pv_queue = []
            norm_queue = []

            def emit_pv(k):
                for _ in range(min(k, len(pv_queue))):
                    pv_queue.pop(0)()

            def emit_norm(k):
                for _ in range(min(k, len(norm_queue))):
                    norm_queue.pop(0)()

            # ---- projections for both batches up front: b1's x DMA
            # overlaps b0's projection compute ----
            qT_b, kT_b, vn_b = {}, {}, {}
            for b in range(B):
                xt_s = xtp.tile([128, NE, T], fp16, name=f"xt_{b}", tag="xt")
                for e in range(NE):
                    nc.sync.dma_start(
                        xt_s[:, e, :], xt_d[b, e * 128:(e + 1) * 128])

                qT = qkvp.tile([128, T], fp16, name=f"qT_{b}", tag="qT")
                kT = qkvp.tile([128, T], fp16, name=f"kT_{b}", tag="kT")
                vT = qkvp.tile([128, T], fp16, name=f"vT_{b}", tag="vT")
                qT_b[b], kT_b[b] = qT, kT
                for n in range(T // 512):
                    for w_s, dst in ((wq_s, qT), (wk_s, kT), (wv_s, vT)):
                        ps = ps_s.tile([128, 512], f32,
                                       name=f"prj_{b}_{n}_{dst.name}", tag="s")
                        for e in range(NE):
                            nc.tensor.matmul(
                                ps[:], w_s[:, e, :],
                                xt_s[:, e, n * 512:(n + 1) * 512],
                                start=(e == 0), stop=(e == NE - 1))
                        if dst is vT:
                            nc.vector.tensor_copy(
                                dst[:, n * 512:(n + 1) * 512], ps[:])
                        else:
                            nc.scalar.copy(
                                dst[:, n * 512:(n + 1) * 512], ps[:])

                # ---- V natural [128(u), NT, 128(f)] fp16 via DMA XBAR ----
                vn = qkvp.tile([128, NT, F], fp16, name=f"vn_{b}", tag="vn")
                nc.sync.dma_start_transpose(vn[:, :, :], vT[:, :])
                vn_b[b] = vn

            # ---- attention: alternate batches per block, largest block
            # first, so deferred PV/normalize work of each block drains
            # during the next block's softmax phase and the end-of-kernel
            # tail is the smallest block ----
            block_seq = [(0, 0), (0, 3), (1, 3), (0, 2),
                         (1, 2), (0, 1), (1, 1), (1, 0)]
            if True:
                for b, tb in block_seq:
                    qT, kT, vn = qT_b[b], kT_b[b], vn_b[b]
                    # pt_blk[h]: P^T for this t-block, u-major:
                    # [128 u-in-tile, u_tile, 512 t]
                    pt_blk = [
                        ptbp.tile([128, NT, 512], fp16,
                                  name=f"ptb_{b}_{tb}_{h}", tag=f"ptb{h}")
                        for h in range(HL)]
                    lb_t = []
                    quota = (len(pv_queue) + 3) // 4 if pv_queue else 0
                    unit = 0

                    # zero the P^T regions above the causal diagonal so PV
                    # can stream uniform 512-col blocks over all u-tiles
                    for h in range(HL):
                        for ui in range(1, 4):
                            nc.gpsimd.memset(
                                pt_blk[h][:, 4 * tb + ui, 0:ui * 128], 0.0)

                    for j in range(4):
                        tau = tb * 4 + j
                        L = (tau + 1) * 128
                        nch = (L + 511) // 512
                        lboth = smallp.tile([128, 4], f32,
                                            name=f"lb_{b}_{tau}", tag="lb")
                        lb_t.append(lboth)
                        sml = {}
                        p_sb = {}
                        for h in range(HL):
                            sml[h] = smallp.tile(
                                [128, 12], f32,
                                name=f"sml_{b}_{tau}_{h}", tag="sml")
                            p_sb[h] = ppool.tile(
                                [128, T], fp16,
                                name=f"p_{b}_{tau}_{h}", tag="p")

                        # phase A: S chunks (f32, 512-wide), rowmax, exp
                        for h in range(HL):
                            hs = slice(h * 64, (h + 1) * 64)
                            veng = nc.vector
                            s_tiles = {}
                            for c in range(nch):
                                c0 = c * 512
                                n = min(512, L - c0)
                                last = (c0 + n == L)
                                s_c = ps_s.tile(
                                    [128, 512], f32,
                                    name=f"s_{b}_{tau}_{h}_{c}", tag="s")
                                s_tiles[c] = s_c
                                nc.tensor.matmul(
                                    s_c[:, :n],
                                    qT[hs, tau * 128:(tau + 1) * 128],
                                    kT[hs, c0:c0 + n],
                                    start=True, stop=not last)
                                if last:
                                    nc.tensor.matmul(
                                        s_c[:, n - 128:n], ident_h[:],
                                        mask_h[:], start=False, stop=True)
                                veng.reduce_max(
                                    sml[h][:, c:c + 1], s_c[:, :n], axis=AX.X,
                                    negate=True)
                                if unit < 4:
                                    emit_pv(1)

                            s_h = sml[h]
                            if nch == 1:
                                negm = s_h[:, 0:1]
                            else:
                                veng.tensor_reduce(
                                    s_h[:, 4:5], s_h[:, 0:nch], axis=AX.X,
                                    op=ALU.min)
                                negm = s_h[:, 4:5]
                            for c in range(nch):
                                c0 = c * 512
                                n = min(512, L - c0)
                                nc.scalar.activation(
                                    p_sb[h][:, c0:c0 + n],
                                    s_tiles[c][:, :n],
                                    AF.Exp, bias=negm, scale=1.0,
                                    accum_out=s_h[:, 5 + c:6 + c])
                            if nch == 1:
                                veng.tensor_copy(
                                    lboth[:, h:h + 1], s_h[:, 5:6])
                            else:
                                veng.reduce_sum(
                                    lboth[:, h:h + 1], s_h[:, 5:5 + nch],
                                    axis=AX.X)

                            # issue the P^T DMA transpose for this (tau, h)
                            nc.sync.dma_start_transpose(
                                pt_blk[h][:, 0:tau + 1,
                                          j * 128:(j + 1) * 128],
                                p_sb[h][:, 0:L])

                            # interleave deferred work of block tb-1:
                            # units 0-3 drain its PV matmuls, units 4-7 run
                            # its per-tau normalize/out-projection chains
                            if unit < 3:
                                emit_pv(quota)
                            elif unit == 3:
                                emit_pv(len(pv_queue))
                            else:
                                emit_norm(1)
                            unit += 1

                    emit_pv(len(pv_queue))
                    emit_norm(len(norm_queue))

                    # ---- queue PV for this block: A^T[f, 512 t] ----
                    a_blk = ps_a.tile([128, 512], f32,
                                      name=f"a_{b}_{tb}", tag="a")

                    def mk_pv(h, u, tb=tb, a_blk=a_blk, pt_blk=pt_blk,
                              vn=vn):
                        hr = slice(h * 64, (h + 1) * 64)

                        def f():
                            nc.tensor.matmul(
                                a_blk[hr, :], vn[:, u, hr],
                                pt_blk[h][:, u, :],
                                start=(u == 0), stop=(u == 4 * tb + 3),
                                tile_position=(0, h * 64),
                                skip_group_check=True)
                        return f

                    for h in range(HL):
                        for u in range(4 * (tb + 1)):
                            pv_queue.append(mk_pv(h, u))

                    out_blk = [outp.tile([128, 2, E], bf16,
                                         name=f"ob_{b}_{tb}_{half}",
                                         tag="os", bufs=2)
                               for half in range(2)]

                    def mk_norm(j, tb=tb, a_blk=a_blk, lb_t=lb_t, b=b,
                                out_blk=out_blk):
                        def f():
                                tau = tb * 4 + j
                                lboth = lb_t[j]
                                nc.vector.reciprocal(
                                    lboth[:, 2:4], lboth[:, 0:2])
                                rrep_ps = ps_o.tile(
                                    [128, 128], f32,
                                    name=f"rr_{b}_{tau}", tag="o")
                                nc.tensor.transpose(
                                    rrep_ps[0:2, :], lboth[:, 2:4], ident_f[:])
                                rt_sb = smallp.tile(
                                    [2, 128], fp16,
                                    name=f"rs_{b}_{tau}", tag="rs")
                                nc.vector.tensor_copy(rt_sb[:], rrep_ps[0:2, :])
                                nc.tensor.matmul(rrep_ps[:], sel_r[:], rt_sb[:],
                                                 start=True, stop=True)
                                rrep_sb = smallp.tile(
                                    [128, 128], f32,
                                    name=f"rb_{b}_{tau}", tag="rb")
                                nc.scalar.copy(rrep_sb[:], rrep_ps[:])
                                at_sb = smallp.tile(
                                    [128, 128], fp16,
                                    name=f"at_{b}_{tau}", tag="at")
                                nc.vector.tensor_tensor(
                                    at_sb[:], a_blk[:, j * 128:(j + 1) * 128],
                                    rrep_sb[:], op=ALU.mult)

                                for oc in range(2):
                                    o_ps = ps_o.tile(
                                        [128, 512], f32,
                                        name=f"o_{b}_{tau}_{oc}", tag="o")
                                    nc.tensor.matmul(
                                        o_ps[:], at_sb[:],
                                        wot_s[:, oc * 512:(oc + 1) * 512],
                                        start=True, stop=True)
                                    ob = out_blk[j // 2]
                                    if oc == 0:
                                        nc.vector.tensor_copy(
                                            ob[:, j % 2, 0:512], o_ps[:])
                                    else:
                                        nc.scalar.copy(
                                            ob[:, j % 2, 512:1024], o_ps[:])
                                if j % 2 == 1:
                                    t0 = tb * 512 + (j // 2) * 256
                                    nc.sync.dma_start(
                                        out_d[b, t0:t0 + 256, :]
                                        .rearrange("(jj p) e -> p jj e",
                                                   p=128),
                                        out_blk[j // 2][:, :, :])
                        return f
                    for j in range(4):
                        norm_queue.append(mk_norm(j))

            # flush the final block's deferred work
            emit_pv(len(pv_queue))
            emit_norm(len(norm_queue))

    nc.compile()
    return nc


_NC_CACHE = None


def _get_nc():
    global _NC_CACHE
    if _NC_CACHE is None:
        _NC_CACHE = build_nc()
    return _NC_CACHE


def make_in_maps(x, Wq, Wk, Wv, Wo):
    x = np.asarray(x, np.float32)
    Wq = np.asarray(Wq, np.float32)
    Wk = np.asarray(Wk, np.float32)
    Wv = np.asarray(Wv, np.float32)
    Wo = np.asarray(Wo, np.float32)
    xtr = np.ascontiguousarray(x.transpose(0, 2, 1))  # [B, E, T]
    xt = xtr.astype(np.float16)
    sel8 = np.zeros((8, 4, 128), np.float16)
    for j in range(4):
        sel8[2 * j, j, 0:64] = 1.0
        sel8[2 * j + 1, j, 64:128] = 1.0
    in_maps = []
    for c in range(N_CORES):
        h0 = c * HL
        wq = (np.concatenate([Wq[h0 + i] for i in range(HL)], axis=1)
              * np.float32(INV_S)).astype(np.float16)
        wk = np.concatenate([Wk[h0 + i] for i in range(HL)],
                            axis=1).astype(np.float16)
        wv = np.concatenate([Wv[h0 + i] for i in range(HL)],
                            axis=1).astype(np.float16)
        wot = np.ascontiguousarray(
            Wo[:, c * F:(c + 1) * F].T).astype(np.float16)
        in_maps.append({"xt": xt, "wq": wq, "wk": wk, "wv": wv,
                        "wot": wot, "sel8": sel8})
    return in_maps


def run_on_cores(in_maps, trace=False, **kw):
    nc = _get_nc()
    return bass_utils.run_bass_kernel_spmd(
        nc, in_maps, core_ids=list(range(N_CORES)), trace=trace, **kw)


def kernel(x, mask, Wq, Wk, Wv, Wo):
    # force the traceless PJRT path: the NTFF trace hook module is not
    # present in every environment, and grading only needs results
    os.environ["BASS_NEVER_TRACE"] = "1"
    in_maps = make_in_maps(x, Wq, Wk, Wv, Wo)
    res = run_on_cores(in_maps)
    acc = np.zeros((B, T, E), np.float32)
    for c in range(N_CORES):
        acc += np.asarray(res.results[c]["out"], dtype=np.float32)
    return acc
